# revision 31
# baseline (speedup 1.0000x reference)
"""Trainium2 Bass kernel for the CapacityNN PINN forward pass.

Computes, for N = B*S collocation points x = (s, t):
  U   = MLP([s_norm, t_norm]) * tgt_std + tgt_mean
  F   = U_t  - G(U)             (G = Verhulst logistic growth term)
  F_t = U_tt - G'(U) * U_t
where U_t/U_tt are 1st/2nd derivatives w.r.t. t_norm, computed exactly by
forward-mode Taylor (jet) propagation through the tanh MLP.

Sharding: pure data parallel over 8 NeuronCores (8192 points/core),
MLP weights + PDE scalars replicated. All math runs on-device; the host
only reorders data (weight packing, shard/gather).

Device layout: hidden dim (256) on partitions (2 halves of 128), points
on the free dim, chunks of 1024 points. Streams per layer (all fp16):
  hv = tanh values, h1 = dU/dt_norm jet, h2 = d2U/dt_norm2 jet.
Layer-0 jets are folded into layer-1 weight copies (rank-1 structure in
t_norm), so layer 0 runs the primal matmul only.

Schedule: chunks start every 2 pipeline slots (l0 | l1 | l2 | l3 | proj),
so each slot mixes layers of 2-3 chunks and engine queues stay fed.
Jet psums are evacuated to fp16 SBUF immediately by the Activation
engine (z1s = sqrt2*z1, z2s = scale*z2; the scales make s1p = z1s^2 the
exact 2*z1^2 term, with stream scales alpha_l = 2^(l/2), beta_l =
2^(l-1) folded into the host-scaled projection weights), so PSUM banks
recycle in ~1us and the jet algebra runs on DVE's fast fp16 SBUF modes
(tensor_scalar 4x, tensor_tensor 2x). GPSIMD takes only off-chain
SBUF ops (it cannot access PSUM on TRN2 hardware). Constants arrive in
a few blob DMAs (x2+W0 fused fp32r; fp32 scalars incl host-computed PDE
terms and folded biases; one fp16 weight blob); PSUM tiles are 2-bank
[128,1024] so psum consumers run 1024 wide.
"""

import os
import sys
import tempfile

import numpy as np

for _p in ("/opt/trn_rl_repo", "/root/.axon_site/_ro/trn_rl_repo"):
    if os.path.isdir(_p) and _p not in sys.path:
        sys.path.insert(0, _p)

import concourse.bass as bass
import concourse.bacc as bacc
import concourse.tile as tile
from concourse import mybir
from concourse.bass_utils import run_bass_kernel_spmd

AF = mybir.ActivationFunctionType
OP = mybir.AluOpType
F32 = mybir.dt.float32
F32R = mybir.dt.float32r
F16 = mybir.dt.float16

NCORES = 8
B, S, H = 512, 128, 256
N = B * S                  # 65536 points
NLOC = N // NCORES         # 8192 points per core
CH = 1024                  # points per on-chip chunk
NCHUNK = NLOC // CH        # 8
PPH = (NLOC // 4) // 128   # 16 points per partition per tail quarter
NC16 = 20 * 128 + 18 + 256  # fp16 const blob cols (incl 2 -I tiles)
SQRT2 = float(np.sqrt(2.0))

# c32 scalar blob column indices
IC_STS, IC_TMB, IC_C, IC_C1, IC_NR, IC_MC3, IC_BETA0, IC_BL = 0, 1, 2, 3, 4, 5, 6, 8


def _build():
    nc = bacc.Bacc(
        "TRN2",
        target_bir_lowering=False,
        debug=False,
        enable_asserts=False,
        num_devices=NCORES,
    )

    xw = nc.dram_tensor("xw", [2, NLOC + H], F32R, kind="ExternalInput").ap()
    c32 = nc.dram_tensor("c32", [128, 16], F32, kind="ExternalInput").ap()
    c16 = nc.dram_tensor("c16", [128, NC16], F16, kind="ExternalInput").ap()
    out = nc.dram_tensor("out", [128, 12 * PPH], F32, kind="ExternalOutput").ap()

    with tile.TileContext(nc) as tc:
        from contextlib import ExitStack

        with ExitStack() as ctx:
            const = ctx.enter_context(tc.tile_pool(name="const", bufs=1))
            sb = ctx.enter_context(tc.tile_pool(name="sb", bufs=1))
            ps = ctx.enter_context(tc.tile_pool(name="ps", bufs=1, space="PSUM"))
            ps2 = ctx.enter_context(tc.tile_pool(name="ps2", bufs=1, space="PSUM"))

            # input DMAs, ordered so compute can start earliest: W0 + scalars,
            # first point-pair, weights, remaining points
            c32_t = const.tile([128, 16], F32, name="c32_t")
            nc.sync.dma_start(out=c32_t, in_=c32)
            xw_t = const.tile([2, NLOC + H], F32R, name="xw_t")
            nc.sync.dma_start(
                out=xw_t[:, NLOC : NLOC + H],
                in_=bass.AP(xw.tensor, NLOC, [[NLOC + H, 2], [1, H]]),
            )
            nc.sync.dma_start(
                out=xw_t[:, 0 : 2 * CH],
                in_=bass.AP(xw.tensor, 0, [[NLOC + H, 2], [1, 2 * CH]]),
            )
            c16_t = const.tile([128, NC16], F16, name="c16_t")
            nc.sync.dma_start(out=c16_t, in_=c16)
            nc.sync.dma_start(
                out=xw_t[:, 2 * CH : NLOC],
                in_=bass.AP(xw.tensor, 2 * CH, [[NLOC + H, 2], [1, NLOC - 2 * CH]]),
            )

            def scal(i):
                return c32_t[:, i : i + 1]

            def w0(m):
                o = NLOC + m * 128
                return xw_t[:, o : o + 128]

            def wt(l, m, kk):
                o = ((l - 1) * 4 + m * 2 + kk) * 128
                return c16_t[:, o : o + 128]

            def w1w(m, kk):
                o = (12 + m * 2 + kk) * 128
                return c16_t[:, o : o + 128]

            def w1w2(m, kk):
                o = (16 + m * 2 + kk) * 128
                return c16_t[:, o : o + 128]

            def lt4(s_idx, kk):
                o = 20 * 128 + (s_idx * 2 + kk) * 3
                return c16_t[:, o : o + 3]

            def nid(l):
                o = 20 * 128 + 18 + (0 if l == 1 else 128)
                return c16_t[:, o : o + 128]

            y3f = sb.tile([3, NLOC], F32, name="y3f")

            def tail(qf):
                """PDE algebra on one quarter of the points; writes out DMA."""
                NQ = NLOC // 4
                base = qf * NQ
                tp = sb.tile([128, 3 * PPH], F32, tag="tp", bufs=2, name="tp")
                for s_idx in range(3):
                    nc.sync.dma_start(
                        out=tp[:, s_idx * PPH : (s_idx + 1) * PPH],
                        in_=y3f[s_idx : s_idx + 1, base : base + NQ],
                    )
                yv = tp[:, 0:PPH]
                yt = tp[:, PPH : 2 * PPH]
                ytt = tp[:, 2 * PPH : 3 * PPH]
                oc = sb.tile([128, 3 * PPH], F32, tag="oc", bufs=2, name="oc")
                U = oc[:, 0:PPH]
                Fo = oc[:, PPH : 2 * PPH]
                Ft = oc[:, 2 * PPH : 3 * PPH]

                def tl(name):
                    return sb.tile([128, PPH], F32, tag=name, bufs=2, name=name)

                ut, utt, vv, v2, w1_, q1, t1 = (
                    tl("ut"), tl("utt"), tl("vv"), tl("v2"),
                    tl("w1_"), tl("q1"), tl("t1"),
                )
                nc.vector.tensor_scalar(U, yv, scal(IC_STS), scal(IC_TMB), OP.mult, OP.add)
                nc.vector.tensor_scalar(ut, yt, scal(IC_STS), None, OP.mult)
                nc.vector.tensor_scalar(utt, ytt, scal(IC_STS), None, OP.mult)
                nc.vector.tensor_scalar(vv, U, scal(IC_C), None, OP.subtract)
                nc.vector.tensor_tensor(v2, vv, vv, OP.mult)
                nc.vector.scalar_tensor_tensor(w1_, v2, scal(IC_C1), vv, OP.mult, OP.add)
                nc.vector.scalar_tensor_tensor(Fo, w1_, scal(IC_NR), ut, OP.mult, OP.add)
                nc.vector.tensor_tensor(q1, vv, ut, OP.mult)
                nc.vector.scalar_tensor_tensor(t1, ut, scal(IC_NR), utt, OP.mult, OP.add)
                nc.vector.scalar_tensor_tensor(Ft, q1, scal(IC_MC3), t1, OP.mult, OP.add)
                nc.sync.dma_start(
                    out=bass.AP(
                        out.tensor, qf * 3 * PPH, [[12 * PPH, 128], [1, 3 * PPH]]
                    ),
                    in_=oc,
                )

            def new_tile(tag, m, bufs=4, w=CH):
                return sb.tile([128, w], F16, tag=f"{tag}{m}", bufs=bufs, name=tag)

            def psum_tile(name):
                return ps.tile([128, 1024], F32, tag="pz", bufs=3, name=name)

            st = [dict() for _ in range(NCHUNK)]  # per-chunk stream state

            def mm_group(pz, lhsT_of, rhs, rhs_off=0, stop_last=True, start_first=True):
                """4 matmuls [128,512] accumulating the two kk halves."""
                for g in range(2):
                    for kk in range(2):
                        nc.tensor.matmul(
                            pz[:, g * 512 : (g + 1) * 512],
                            lhsT_of(kk),
                            rhs[kk][:, rhs_off + g * 512 : rhs_off + (g + 1) * 512],
                            start=(kk == 0) and start_first,
                            stop=(kk == 1) and stop_last,
                        )

            def stage_l0(c):
                """Layer 0: primal tanh + jet seeds (dm0, av0*dm0)."""
                hv = [new_tile("hv", m) for m in range(2)]
                ee = [new_tile("ee", m, bufs=2) for m in range(2)]
                dm = [new_tile("dm", m, bufs=4) for m in range(2)]
                h2 = [new_tile("h2", m) for m in range(2)]
                for m in range(2):
                    pz = psum_tile("pz0")
                    for g in range(2):
                        nc.tensor.matmul(
                            pz[:, g * 512 : (g + 1) * 512],
                            w0(m),
                            xw_t[:, c * CH + g * 512 : c * CH + (g + 1) * 512],
                            start=True,
                            stop=True,
                        )
                    nc.scalar.activation(hv[m], pz, AF.Tanh, scal(IC_BETA0 + m))
                nc.vector.tensor_tensor(ee[1], hv[1], hv[1], OP.mult)
                nc.vector.tensor_scalar(dm[1], ee[1], -1.0, 1.0, OP.mult, OP.add)
                nc.gpsimd.tensor_tensor(ee[0], hv[0], hv[0], OP.mult)
                nc.vector.tensor_tensor(h2[1], hv[1], dm[1], OP.mult)
                nc.vector.tensor_scalar(dm[0], ee[0], -1.0, 1.0, OP.mult, OP.add)
                nc.gpsimd.tensor_tensor(h2[0], hv[0], dm[0], OP.mult)
                st[c]["hv"], st[c]["h1"], st[c]["h2"] = hv, dm, h2

            def stage_hidden(c, l):
                """One hidden layer for one chunk, psum-evacuation dataflow.

                Jet psums are evacuated to fp16 SBUF right away (z1s by Act
                with scale sqrt2; z2 by Act for m=1 / DVE for m=0 with the
                layer's scale), so psum banks recycle fast and downstream jet
                algebra runs on fp16 SBUF tiles. Stream scales
                alpha_l = 2^(l/2), beta_l = 2^(l-1) fold into the projection.
                """
                hv, h1, h2 = st[c]["hv"], st[c]["h1"], st[c]["h2"]
                hv_n = [new_tile("hv", m) for m in range(2)]
                h1_n = [new_tile("h1", m) for m in range(2)]
                h2_n = [new_tile("h2", m) for m in range(2)]
                ee = [new_tile("ee", m, bufs=2) for m in range(2)]
                dm = [new_tile("dm", m, bufs=4) for m in range(2)]
                z1s = [new_tile("z1s", m, bufs=3) for m in range(2)]
                s1p = [new_tile("s1p", m, bufs=2) for m in range(2)]
                tt2 = [new_tile("tt2", m, bufs=4) for m in range(2)]
                z2s = [new_tile("z2s", m, bufs=3) for m in range(2)]
                qq = [new_tile("qq", m, bufs=2) for m in range(2)]

                def w1(m, kk):
                    return w1w(m, kk) if l == 1 else wt(l, m, kk)

                def w2(m, kk):
                    return w1w2(m, kk) if l == 1 else wt(l, m, kk)

                z2scale = 1.0 if l == 1 else 2.0
                bias = lambda m: scal(IC_BL + 2 * (l - 1) + m)

                for m in range(2):
                    pz = psum_tile("pzP")
                    mm_group(pz, lambda kk, _m=m: wt(l, _m, kk), hv)
                    nc.scalar.activation(hv_n[m], pz, AF.Tanh, bias(m))
                nc.vector.tensor_tensor(ee[1], hv_n[1], hv_n[1], OP.mult)
                nc.vector.tensor_scalar(dm[1], ee[1], -1.0, 1.0, OP.mult, OP.add)
                nc.gpsimd.tensor_tensor(ee[0], hv_n[0], hv_n[0], OP.mult)

                pz1 = psum_tile("pz1")
                mm_group(pz1, lambda kk: w1(1, kk), h1)
                nc.scalar.mul(z1s[1], pz1, SQRT2)
                nc.vector.tensor_tensor(h1_n[1], dm[1], z1s[1], OP.mult)
                pz1 = psum_tile("pz1")
                mm_group(pz1, lambda kk: w1(0, kk), h1)
                nc.scalar.mul(z1s[0], pz1, SQRT2)
                nc.vector.tensor_scalar(dm[0], ee[0], -1.0, 1.0, OP.mult, OP.add)
                nc.vector.tensor_tensor(h1_n[0], dm[0], z1s[0], OP.mult)
                nc.vector.tensor_tensor(s1p[1], z1s[1], z1s[1], OP.mult)
                nc.vector.tensor_tensor(tt2[1], hv_n[1], s1p[1], OP.mult)
                nc.gpsimd.tensor_tensor(s1p[0], z1s[0], z1s[0], OP.mult)
                nc.vector.tensor_tensor(tt2[0], hv_n[0], s1p[0], OP.mult)

                pz2 = psum_tile("pz2")
                mm_group(pz2, lambda kk: w2(1, kk), h2)
                nc.scalar.mul(z2s[1], pz2, z2scale)
                nc.vector.tensor_tensor(qq[1], z2s[1], tt2[1], OP.subtract)
                nc.vector.tensor_tensor(h2_n[1], dm[1], qq[1], OP.mult)
                pz2 = psum_tile("pz2")
                mm_group(pz2, lambda kk: w2(0, kk), h2)
                nc.scalar.mul(z2s[0], pz2, z2scale)
                nc.vector.tensor_tensor(qq[0], z2s[0], tt2[0], OP.subtract)
                nc.vector.tensor_tensor(h2_n[0], dm[0], qq[0], OP.mult)
                st[c]["hv"], st[c]["h1"], st[c]["h2"] = hv_n, h1_n, h2_n

            def stage_proj(c):
                hv, h1, h2 = st[c]["hv"], st[c]["h1"], st[c]["h2"]
                for i in range(CH // 512):
                    py = ps2.tile([3, 512], F32, tag="py", bufs=2, name="py")
                    first = True
                    for s_idx, stream in enumerate((hv, h1, h2)):
                        for kk in range(2):
                            nc.tensor.matmul(
                                py,
                                lt4(s_idx, kk),
                                stream[kk][:, i * 512 : (i + 1) * 512],
                                start=first,
                                stop=(s_idx == 2 and kk == 1),
                            )
                            first = False
                    nc.vector.tensor_scalar(
                        y3f[:, c * CH + i * 512 : c * CH + (i + 1) * 512], py,
                        1.0, None, OP.mult,
                    )

            def stage(c, s):
                if s == 0:
                    stage_l0(c)
                elif s <= 3:
                    stage_hidden(c, s)
                else:
                    stage_proj(c)
                    if c % 2 == 1:
                        tail(c // 2)

            # software pipeline: chunk c runs stages at slots 2c .. 2c+4,
            # so each slot mixes different layers of 2-3 chunks
            NSLOT = 2 * (NCHUNK - 1) + 5
            for k in range(NSLOT):
                for c in range(NCHUNK):
                    s = k - 2 * c
                    if 0 <= s <= 4:
                        stage(c, s)

    nc.compile()
    return nc


_STATE = {}


def _get_nc():
    if "nc" not in _STATE:
        _STATE["nc"] = _build()
    return _STATE["nc"]


def _sigmoid(x):
    return 1.0 / (1.0 + np.exp(-x))


def _prep_in_maps(inputs):
    f = np.float32

    def arr(k):
        return np.asarray(inputs[k], f)

    x = np.asarray(inputs["inputs"], f).reshape(N, 2)
    W0, b0 = arr("W0"), arr("b0")
    W1, W2, W3 = arr("W1"), arr("W2"), arr("W3")
    W4, b4 = arr("W4").reshape(1, H), arr("b4").reshape(1)
    in_mean, in_std = arr("in_mean"), arr("in_std")
    tgt_mean, tgt_std = arr("tgt_mean"), arr("tgt_std")

    # PDE scalars (host-computed, replicated)
    r = np.exp(-arr("log_growth_rate"))
    K = 0.2 + 0.8 * _sigmoid(arr("log_carrying_capacity"))
    C = 0.1 * _sigmoid(arr("log_initial_loss"))
    ikc = 1.0 / (K - C)

    inv_std = 1.0 / (in_std + 1e-8)
    w0ts = (W0 * inv_std[None, :]).T.astype(f)          # [2, H]
    beta0 = b0 - W0 @ (in_mean * inv_std)               # [H]

    c32 = np.zeros((128, 16), f)
    c32[:, IC_STS] = tgt_std[0]
    c32[:, IC_TMB] = b4[0] * tgt_std[0] + tgt_mean[0]
    c32[:, IC_C] = C
    c32[:, IC_C1] = -ikc
    c32[:, IC_NR] = -r
    c32[:, IC_MC3] = 2.0 * r * ikc
    for m in range(2):
        c32[:, IC_BETA0 + m] = beta0[m * 128 : (m + 1) * 128]
    for li, bl in enumerate((arr("b1"), arr("b2"), arr("b3"))):
        for m in range(2):
            c32[:, IC_BL + 2 * li + m] = bl[m * 128 : (m + 1) * 128]

    w0c1 = W0[:, 1]
    A1 = (W1 * w0c1[None, :]).T                          # (W1 diag(w0c1))^T
    A2 = (W1 * (-2.0 * w0c1 ** 2)[None, :]).T
    c16 = np.zeros((128, NC16), np.float16)
    for l, Wl in ((1, W1), (2, W2), (3, W3)):
        WT = Wl.T
        for m in range(2):
            for kk in range(2):
                o = ((l - 1) * 4 + m * 2 + kk) * 128
                c16[:, o : o + 128] = WT[kk * 128 : (kk + 1) * 128, m * 128 : (m + 1) * 128]
    for base, A in ((12, A1), (16, A2)):
        for m in range(2):
            for kk in range(2):
                o = (base + m * 2 + kk) * 128
                c16[:, o : o + 128] = A[kk * 128 : (kk + 1) * 128, m * 128 : (m + 1) * 128]
    o = 20 * 128 + 18
    c16[:, o : o + 128] = -np.eye(128, dtype=np.float16)
    c16[:, o + 128 : o + 256] = -0.5 * np.eye(128, dtype=np.float16)
    # stream scales from the on-device jet-psum evacuation:
    # h1 carries alpha_3 = 2^(3/2), h2 carries beta_3 = 4
    sscale = (1.0, 2.0 ** -1.5, 0.25)
    for s_idx in range(3):
        for kk in range(2):
            o = 20 * 128 + (s_idx * 2 + kk) * 3
            c16[:, o + s_idx] = W4[0, kk * 128 : (kk + 1) * 128] * sscale[s_idx]

    shared = {"c32": c32, "c16": c16}
    in_maps = []
    for c in range(NCORES):
        m = dict(shared)
        xwc = np.zeros((2, NLOC + H), f)
        xwc[:, :NLOC] = x[c * NLOC : (c + 1) * NLOC].T
        xwc[:, NLOC:] = w0ts
        m["xw"] = xwc
        in_maps.append(m)
    return in_maps


def _decode_out(o):
    """[128, 12*PPH] device layout -> (U, F, Ft) flat [NLOC] arrays."""
    a = o.reshape(128, 4, 3, PPH)
    res = []
    for s_idx in range(3):
        res.append(a[:, :, s_idx, :].transpose(1, 0, 2).reshape(NLOC))
    return res


def run(inputs, trace=False):
    nc = _get_nc()
    in_maps = _prep_in_maps(inputs)
    kw = {}
    if trace:
        kw["tmpdir"] = tempfile.mkdtemp(prefix="bassk_prof_")
    res = run_bass_kernel_spmd(
        nc, in_maps, core_ids=list(range(NCORES)), trace=trace, **kw
    )
    U = np.empty((N,), np.float32)
    F = np.empty((N,), np.float32)
    Ft = np.empty((N,), np.float32)
    for c in range(NCORES):
        u, ff, ft = _decode_out(res.results[c]["out"])
        U[c * NLOC : (c + 1) * NLOC] = u
        F[c * NLOC : (c + 1) * NLOC] = ff
        Ft[c * NLOC : (c + 1) * NLOC] = ft
    shp = (B, S, 1)
    return (U.reshape(shp), F.reshape(shp), Ft.reshape(shp)), res


def kernel(**inputs):
    outs, _ = run(inputs, trace=False)
    return outs


# ---------------------------------------------------------------------------
# Dev-loop timing: persistent jitted executable (mirrors
# bass2jax.run_bass_via_pjrt's multi-core branch) so repeated executions
# reuse one compiled NEFF and can be timed back-to-back.
# ---------------------------------------------------------------------------
def _make_runner():
    if "runner" in _STATE:
        return _STATE["runner"]
    import jax
    from jax.experimental.shard_map import shard_map
    from jax.sharding import Mesh, PartitionSpec
    from concourse import bass2jax

    bass2jax.install_neuronx_cc_hook()
    nc = _get_nc()

    in_names, out_names, out_avals, zero_outs = [], [], [], []
    for alloc in nc.m.functions[0].allocations:
        if not isinstance(alloc, mybir.MemoryLocationSet):
            continue
        name = alloc.memorylocations[0].name
        if alloc.kind == "ExternalInput":
            if nc.partition_id_tensor is None or name != nc.partition_id_tensor.name:
                in_names.append(name)
        elif alloc.kind == "ExternalOutput":
            out_names.append(name)
            shape = tuple(alloc.tensor_shape)
            dtype = mybir.dt.np(alloc.dtype)
            out_avals.append(jax.core.ShapedArray(shape, dtype))
            zero_outs.append(np.zeros(shape, dtype))
    n_params = len(in_names)
    n_outs = len(out_avals)
    all_names = in_names + out_names
    if nc.partition_id_tensor is not None:
        all_names = all_names + [nc.partition_id_tensor.name]

    def _body(*args):
        operands = list(args)
        if nc.partition_id_tensor is not None:
            operands.append(bass2jax.partition_id_tensor())
        outs = bass2jax._bass_exec_p.bind(
            *operands,
            out_avals=tuple(out_avals),
            in_names=tuple(all_names),
            out_names=tuple(out_names),
            lowering_input_output_aliases=(),
            sim_require_finite=True,
            sim_require_nnan=True,
            nc=nc,
        )
        return tuple(outs)

    devices = jax.devices()[:NCORES]
    mesh = Mesh(np.asarray(devices), ("core",))
    donate = tuple(range(n_params, n_params + n_outs))
    sharded = jax.jit(
        shard_map(
            _body,
            mesh=mesh,
            in_specs=(PartitionSpec("core"),) * (n_params + n_outs),
            out_specs=(PartitionSpec("core"),) * n_outs,
            check_rep=False,
        ),
        donate_argnums=donate,
        keep_unused=True,
    )
    _STATE["runner"] = (sharded, in_names, out_names, out_avals, zero_outs)
    return _STATE["runner"]


def run_timed(inputs, iters=20):
    """Run via a persistent executable; return (outputs, per_iter_ns)."""
    import time as _time

    import jax

    sharded, in_names, out_names, out_avals, zero_outs = _make_runner()
    in_maps = _prep_in_maps(inputs)
    concat_in = [
        np.concatenate([np.asarray(in_maps[c][n]) for c in range(NCORES)], axis=0)
        for n in in_names
    ]
    dev_in = [jax.device_put(a) for a in concat_in]

    def zeros():
        return [
            np.zeros((NCORES * z.shape[0], *z.shape[1:]), z.dtype) for z in zero_outs
        ]

    # warmup (compiles on first call)
    outs = sharded(*dev_in, *zeros())
    jax.block_until_ready(outs)
    out_np = [np.asarray(o) for o in outs]

    zbufs = [zeros() for _ in range(iters)]
    t0 = _time.perf_counter()
    last = None
    for i in range(iters):
        last = sharded(*dev_in, *zbufs[i])
    jax.block_until_ready(last)
    t1 = _time.perf_counter()
    per_iter_ns = (t1 - t0) / iters * 1e9

    per_core = [
        {
            name: out_np[i].reshape(NCORES, *out_avals[i].shape)[c]
            for i, name in enumerate(out_names)
        }
        for c in range(NCORES)
    ]
    U = np.empty((N,), np.float32)
    F = np.empty((N,), np.float32)
    Ft = np.empty((N,), np.float32)
    for c in range(NCORES):
        u, ff, ft = _decode_out(per_core[c]["out"])
        U[c * NLOC : (c + 1) * NLOC] = u
        F[c * NLOC : (c + 1) * NLOC] = ff
        Ft[c * NLOC : (c + 1) * NLOC] = ft
    shp = (B, S, 1)
    return (U.reshape(shp), F.reshape(shp), Ft.reshape(shp)), per_iter_ns


# revision 37
# speedup vs baseline: 1.0306x; 1.0306x over previous
"""Trainium2 Bass kernel for the CapacityNN PINN forward pass.

Computes, for N = B*S collocation points x = (s, t):
  U   = MLP([s_norm, t_norm]) * tgt_std + tgt_mean
  F   = U_t  - G(U)             (G = Verhulst logistic growth term)
  F_t = U_tt - G'(U) * U_t
where U_t/U_tt are 1st/2nd derivatives w.r.t. t_norm, computed exactly by
forward-mode Taylor (jet) propagation through the tanh MLP.

Sharding: pure data parallel over 8 NeuronCores (8192 points/core),
MLP weights + PDE scalars replicated. All math runs on-device; the host
only reorders data (weight packing, shard/gather).

Device layout: hidden dim (256) on partitions (2 halves of 128), points
on the free dim, chunks of 1024 points. Streams per layer (all fp16):
  hv = tanh values, h1 = dU/dt_norm jet, h2 = d2U/dt_norm2 jet.
Layer-0 jets are folded into layer-1 weight copies (rank-1 structure in
t_norm), so layer 0 runs the primal matmul only.

All constants arrive in 3 DMAs (x2+W0 fused fp32r, fp32 scalar blob with
host-precomputed PDE scalars, one fp16 weight blob); PSUM tiles are
2-bank [128,1024] so elementwise consumers run 1024 wide.
"""

import os
import sys
import tempfile

import numpy as np

for _p in ("/opt/trn_rl_repo", "/root/.axon_site/_ro/trn_rl_repo"):
    if os.path.isdir(_p) and _p not in sys.path:
        sys.path.insert(0, _p)

import concourse.bass as bass
import concourse.bacc as bacc
import concourse.tile as tile
from concourse import mybir
from concourse.bass_utils import run_bass_kernel_spmd

AF = mybir.ActivationFunctionType
OP = mybir.AluOpType
F32 = mybir.dt.float32
F32R = mybir.dt.float32r
F16 = mybir.dt.float16

NCORES = 8
B, S, H = 512, 128, 256
N = B * S                  # 65536 points
NLOC = N // NCORES         # 8192 points per core
CH = 1024                  # points per on-chip chunk
NCHUNK = NLOC // CH        # 8
PPH = (NLOC // 4) // 128   # 16 points per partition per tail quarter
NC16 = 20 * 128 + 18 + 256  # fp16 const blob cols (incl 2 -I tiles)
SQRT2 = float(np.sqrt(2.0))

# c32 scalar blob column indices
IC_STS, IC_TMB, IC_C, IC_C1, IC_NR, IC_MC3, IC_BETA0, IC_BL = 0, 1, 2, 3, 4, 5, 6, 8


def _build():
    nc = bacc.Bacc(
        "TRN2",
        target_bir_lowering=False,
        debug=False,
        enable_asserts=False,
        num_devices=NCORES,
    )

    xw = nc.dram_tensor("xw", [2, NLOC + H], F32R, kind="ExternalInput").ap()
    c32 = nc.dram_tensor("c32", [128, 16], F32, kind="ExternalInput").ap()
    c16 = nc.dram_tensor("c16", [128, NC16], F16, kind="ExternalInput").ap()
    out = nc.dram_tensor("out", [128, 12 * PPH], F32, kind="ExternalOutput").ap()

    with tile.TileContext(nc) as tc:
        from contextlib import ExitStack

        with ExitStack() as ctx:
            const = ctx.enter_context(tc.tile_pool(name="const", bufs=1))
            sb = ctx.enter_context(tc.tile_pool(name="sb", bufs=1))
            ps = ctx.enter_context(tc.tile_pool(name="ps", bufs=1, space="PSUM"))
            ps2 = ctx.enter_context(tc.tile_pool(name="ps2", bufs=1, space="PSUM"))

            # input DMAs, ordered so compute can start earliest: W0 + scalars,
            # first point-pair, weights, remaining points
            c32_t = const.tile([128, 16], F32, name="c32_t")
            nc.sync.dma_start(out=c32_t, in_=c32)
            w0_t = const.tile([2, H], F32R, name="w0_t")
            nc.sync.dma_start(
                out=w0_t,
                in_=bass.AP(xw.tensor, NLOC, [[NLOC + H, 2], [1, H]]),
            )
            xq = {}

            def load_x(c):
                t = sb.tile([2, CH], F32R, tag="xq", bufs=4, name="xq")
                nc.sync.dma_start(
                    out=t,
                    in_=bass.AP(xw.tensor, c * CH, [[NLOC + H, 2], [1, CH]]),
                )
                xq[c] = t

            load_x(0)
            c16_t = const.tile([128, NC16], F16, name="c16_t")
            nc.sync.dma_start(out=c16_t, in_=c16)

            def scal(i):
                return c32_t[:, i : i + 1]

            def w0(m):
                return w0_t[:, m * 128 : (m + 1) * 128]

            def wt(l, m, kk):
                o = ((l - 1) * 4 + m * 2 + kk) * 128
                return c16_t[:, o : o + 128]

            def w1w(m, kk):
                o = (12 + m * 2 + kk) * 128
                return c16_t[:, o : o + 128]

            def w1w2(m, kk):
                o = (16 + m * 2 + kk) * 128
                return c16_t[:, o : o + 128]

            def lt4(s_idx, kk):
                o = 20 * 128 + (s_idx * 2 + kk) * 3
                return c16_t[:, o : o + 3]

            def nid(l):
                o = 20 * 128 + 18 + (0 if l == 1 else 128)
                return c16_t[:, o : o + 128]

            y3f = sb.tile([3, NLOC], F32, name="y3f")

            tp_live = [None]

            def tail_dma(c):
                """Reshape chunk c's y3f rows into the tail tile (per-chunk,
                so the final quarter only waits on its last chunk)."""
                if c % 2 == 0:
                    tp_live[0] = sb.tile([128, 3 * PPH], F32, tag="tp", bufs=2, name="tp")
                tp = tp_live[0]
                pb = (c % 2) * 64
                for s_idx in range(3):
                    nc.sync.dma_start(
                        out=tp[pb : pb + 64, s_idx * PPH : (s_idx + 1) * PPH],
                        in_=y3f[s_idx : s_idx + 1, c * CH : (c + 1) * CH],
                    )

            def tail(qf):
                """PDE algebra on one quarter of the points; writes out DMA."""
                tp = tp_live[0]
                yv = tp[:, 0:PPH]
                yt = tp[:, PPH : 2 * PPH]
                ytt = tp[:, 2 * PPH : 3 * PPH]
                oc = sb.tile([128, 3 * PPH], F32, tag="oc", bufs=2, name="oc")
                U = oc[:, 0:PPH]
                Fo = oc[:, PPH : 2 * PPH]
                Ft = oc[:, 2 * PPH : 3 * PPH]

                def tl(name):
                    return sb.tile([128, PPH], F32, tag=name, bufs=2, name=name)

                ut, utt, vv, v2, w1_, q1, t1 = (
                    tl("ut"), tl("utt"), tl("vv"), tl("v2"),
                    tl("w1_"), tl("q1"), tl("t1"),
                )
                nc.vector.tensor_scalar(U, yv, scal(IC_STS), scal(IC_TMB), OP.mult, OP.add)
                nc.vector.tensor_scalar(ut, yt, scal(IC_STS), None, OP.mult)
                nc.vector.tensor_scalar(utt, ytt, scal(IC_STS), None, OP.mult)
                nc.vector.tensor_scalar(vv, U, scal(IC_C), None, OP.subtract)
                nc.vector.tensor_tensor(v2, vv, vv, OP.mult)
                nc.vector.scalar_tensor_tensor(w1_, v2, scal(IC_C1), vv, OP.mult, OP.add)
                nc.vector.scalar_tensor_tensor(Fo, w1_, scal(IC_NR), ut, OP.mult, OP.add)
                nc.vector.tensor_tensor(q1, vv, ut, OP.mult)
                nc.vector.scalar_tensor_tensor(t1, ut, scal(IC_NR), utt, OP.mult, OP.add)
                nc.vector.scalar_tensor_tensor(Ft, q1, scal(IC_MC3), t1, OP.mult, OP.add)
                nc.sync.dma_start(
                    out=bass.AP(
                        out.tensor, qf * 3 * PPH, [[12 * PPH, 128], [1, 3 * PPH]]
                    ),
                    in_=oc,
                )

            def new_tile(tag, m, bufs=5, w=CH):
                return sb.tile([128, w], F16, tag=f"{tag}{m}", bufs=bufs, name=tag)

            def psum_tile(name):
                return ps.tile([128, 1024], F32, tag="pz", bufs=3, name=name)

            st = [dict() for _ in range(NCHUNK)]  # per-chunk stream state

            def mm_group(pz, lhsT_of, rhs, rhs_off=0, stop_last=True, start_first=True):
                """4 matmuls [128,512] accumulating the two kk halves."""
                for g in range(2):
                    for kk in range(2):
                        nc.tensor.matmul(
                            pz[:, g * 512 : (g + 1) * 512],
                            lhsT_of(kk),
                            rhs[kk][:, rhs_off + g * 512 : rhs_off + (g + 1) * 512],
                            start=(kk == 0) and start_first,
                            stop=(kk == 1) and stop_last,
                        )

            def stage_l0(c):
                """Layer 0: primal tanh + jet seeds (dm0, av0*dm0)."""
                hv = [new_tile("hv", m) for m in range(2)]
                ee = [new_tile("ee", m, bufs=2) for m in range(2)]
                dm = [new_tile("dm", m, bufs=4) for m in range(2)]
                h2 = [new_tile("h2", m) for m in range(2)]
                for m in range(2):
                    pz = psum_tile("pz0")
                    for g in range(2):
                        nc.tensor.matmul(
                            pz[:, g * 512 : (g + 1) * 512],
                            w0(m),
                            xq[c][:, g * 512 : (g + 1) * 512],
                            start=True,
                            stop=True,
                        )
                    nc.scalar.activation(hv[m], pz, AF.Tanh, scal(IC_BETA0 + m))
                nc.vector.tensor_tensor(ee[1], hv[1], hv[1], OP.mult)
                nc.vector.tensor_scalar(dm[1], ee[1], -1.0, 1.0, OP.mult, OP.add)
                nc.gpsimd.tensor_tensor(ee[0], hv[0], hv[0], OP.mult)
                nc.vector.tensor_tensor(h2[1], hv[1], dm[1], OP.mult)
                nc.vector.tensor_scalar(dm[0], ee[0], -1.0, 1.0, OP.mult, OP.add)
                nc.gpsimd.tensor_tensor(h2[0], hv[0], dm[0], OP.mult)
                st[c]["hv"], st[c]["h1"], st[c]["h2"] = hv, dm, h2

            def stage_hidden(c, l):
                """One hidden layer for one chunk, psum-evacuation dataflow.

                Jet psums are evacuated to fp16 SBUF right away (z1s by Act
                with scale sqrt2; z2 by Act for m=1 / DVE for m=0 with the
                layer's scale), so psum banks recycle fast and downstream jet
                algebra runs on fp16 SBUF tiles. Stream scales
                alpha_l = 2^(l/2), beta_l = 2^(l-1) fold into the projection.
                """
                hv, h1, h2 = st[c]["hv"], st[c]["h1"], st[c]["h2"]
                hv_n = [new_tile("hv", m) for m in range(2)]
                h1_n = [new_tile("h1", m) for m in range(2)]
                h2_n = [new_tile("h2", m) for m in range(2)]
                ee = [new_tile("ee", m, bufs=2) for m in range(2)]
                dm = [new_tile("dm", m, bufs=4) for m in range(2)]
                z1s = [new_tile("z1s", m, bufs=3) for m in range(2)]
                s1p = [new_tile("s1p", m, bufs=2) for m in range(2)]
                tt2 = [new_tile("tt2", m, bufs=4) for m in range(2)]
                z2s = [new_tile("z2s", m, bufs=3) for m in range(2)]
                qq = [new_tile("qq", m, bufs=2) for m in range(2)]

                def w1(m, kk):
                    return w1w(m, kk) if l == 1 else wt(l, m, kk)

                def w2(m, kk):
                    return w1w2(m, kk) if l == 1 else wt(l, m, kk)

                z2scale = 1.0 if l == 1 else 2.0
                bias = lambda m: scal(IC_BL + 2 * (l - 1) + m)

                for m in range(2):
                    pz = psum_tile("pzP")
                    mm_group(pz, lambda kk, _m=m: wt(l, _m, kk), hv)
                    nc.scalar.activation(hv_n[m], pz, AF.Tanh, bias(m))
                nc.vector.tensor_tensor(ee[1], hv_n[1], hv_n[1], OP.mult)
                nc.vector.tensor_scalar(dm[1], ee[1], -1.0, 1.0, OP.mult, OP.add)
                nc.gpsimd.tensor_tensor(ee[0], hv_n[0], hv_n[0], OP.mult)

                pz1 = psum_tile("pz1")
                mm_group(pz1, lambda kk: w1(1, kk), h1)
                nc.scalar.mul(z1s[1], pz1, SQRT2)
                nc.vector.tensor_tensor(h1_n[1], dm[1], z1s[1], OP.mult)
                pz1 = psum_tile("pz1")
                mm_group(pz1, lambda kk: w1(0, kk), h1)
                nc.scalar.mul(z1s[0], pz1, SQRT2)
                nc.vector.tensor_scalar(dm[0], ee[0], -1.0, 1.0, OP.mult, OP.add)
                nc.vector.tensor_tensor(h1_n[0], dm[0], z1s[0], OP.mult)
                nc.vector.tensor_tensor(s1p[1], z1s[1], z1s[1], OP.mult)
                nc.vector.tensor_tensor(tt2[1], hv_n[1], s1p[1], OP.mult)
                nc.gpsimd.tensor_tensor(s1p[0], z1s[0], z1s[0], OP.mult)
                nc.vector.tensor_tensor(tt2[0], hv_n[0], s1p[0], OP.mult)

                pz2 = psum_tile("pz2")
                mm_group(pz2, lambda kk: w2(1, kk), h2)
                nc.scalar.mul(z2s[1], pz2, z2scale)
                nc.vector.tensor_tensor(qq[1], z2s[1], tt2[1], OP.subtract)
                nc.vector.tensor_tensor(h2_n[1], dm[1], qq[1], OP.mult)
                pz2 = psum_tile("pz2")
                mm_group(pz2, lambda kk: w2(0, kk), h2)
                nc.scalar.mul(z2s[0], pz2, z2scale)
                nc.vector.tensor_tensor(qq[0], z2s[0], tt2[0], OP.subtract)
                nc.vector.tensor_tensor(h2_n[0], dm[0], qq[0], OP.mult)
                st[c]["hv"], st[c]["h1"], st[c]["h2"] = hv_n, h1_n, h2_n

            def stage_proj(c):
                hv, h1, h2 = st[c]["hv"], st[c]["h1"], st[c]["h2"]
                for i in range(CH // 512):
                    py = ps2.tile([3, 512], F32, tag="py", bufs=2, name="py")
                    first = True
                    for s_idx, stream in enumerate((hv, h1, h2)):
                        for kk in range(2):
                            nc.tensor.matmul(
                                py,
                                lt4(s_idx, kk),
                                stream[kk][:, i * 512 : (i + 1) * 512],
                                start=first,
                                stop=(s_idx == 2 and kk == 1),
                            )
                            first = False
                    nc.scalar.copy(
                        y3f[:, c * CH + i * 512 : c * CH + (i + 1) * 512], py
                    )

            def stage(c, s):
                if s == 0:
                    stage_l0(c)
                elif s <= 3:
                    stage_hidden(c, s)
                else:
                    stage_proj(c)
                    tail_dma(c)
                    if c % 2 == 1:
                        tail(c // 2)

            # software pipeline: chunk c runs stages at slots 2c .. 2c+4,
            # so each slot mixes different layers of 2-3 chunks
            NSLOT = 2 * (NCHUNK - 1) + 5
            for k in range(NSLOT):
                cpre = (k + 2) // 2
                if k % 2 == 0 and cpre < NCHUNK and cpre not in xq:
                    load_x(cpre)
                for c in range(NCHUNK):
                    s = k - 2 * c
                    if 0 <= s <= 4:
                        stage(c, s)

    nc.compile()
    return nc


_STATE = {}


def _get_nc():
    if "nc" not in _STATE:
        _STATE["nc"] = _build()
    return _STATE["nc"]


def _sigmoid(x):
    return 1.0 / (1.0 + np.exp(-x))


def _prep_in_maps(inputs):
    f = np.float32

    def arr(k):
        return np.asarray(inputs[k], f)

    x = np.asarray(inputs["inputs"], f).reshape(N, 2)
    W0, b0 = arr("W0"), arr("b0")
    W1, W2, W3 = arr("W1"), arr("W2"), arr("W3")
    W4, b4 = arr("W4").reshape(1, H), arr("b4").reshape(1)
    in_mean, in_std = arr("in_mean"), arr("in_std")
    tgt_mean, tgt_std = arr("tgt_mean"), arr("tgt_std")

    # PDE scalars (host-computed, replicated)
    r = np.exp(-arr("log_growth_rate"))
    K = 0.2 + 0.8 * _sigmoid(arr("log_carrying_capacity"))
    C = 0.1 * _sigmoid(arr("log_initial_loss"))
    ikc = 1.0 / (K - C)

    inv_std = 1.0 / (in_std + 1e-8)
    w0ts = (W0 * inv_std[None, :]).T.astype(f)          # [2, H]
    beta0 = b0 - W0 @ (in_mean * inv_std)               # [H]

    c32 = np.zeros((128, 16), f)
    c32[:, IC_STS] = tgt_std[0]
    c32[:, IC_TMB] = b4[0] * tgt_std[0] + tgt_mean[0]
    c32[:, IC_C] = C
    c32[:, IC_C1] = -ikc
    c32[:, IC_NR] = -r
    c32[:, IC_MC3] = 2.0 * r * ikc
    for m in range(2):
        c32[:, IC_BETA0 + m] = beta0[m * 128 : (m + 1) * 128]
    for li, bl in enumerate((arr("b1"), arr("b2"), arr("b3"))):
        for m in range(2):
            c32[:, IC_BL + 2 * li + m] = bl[m * 128 : (m + 1) * 128]

    w0c1 = W0[:, 1]
    A1 = (W1 * w0c1[None, :]).T                          # (W1 diag(w0c1))^T
    A2 = (W1 * (-2.0 * w0c1 ** 2)[None, :]).T
    c16 = np.zeros((128, NC16), np.float16)
    for l, Wl in ((1, W1), (2, W2), (3, W3)):
        WT = Wl.T
        for m in range(2):
            for kk in range(2):
                o = ((l - 1) * 4 + m * 2 + kk) * 128
                c16[:, o : o + 128] = WT[kk * 128 : (kk + 1) * 128, m * 128 : (m + 1) * 128]
    for base, A in ((12, A1), (16, A2)):
        for m in range(2):
            for kk in range(2):
                o = (base + m * 2 + kk) * 128
                c16[:, o : o + 128] = A[kk * 128 : (kk + 1) * 128, m * 128 : (m + 1) * 128]
    o = 20 * 128 + 18
    c16[:, o : o + 128] = -np.eye(128, dtype=np.float16)
    c16[:, o + 128 : o + 256] = -0.5 * np.eye(128, dtype=np.float16)
    # stream scales from the on-device jet-psum evacuation:
    # h1 carries alpha_3 = 2^(3/2), h2 carries beta_3 = 4
    sscale = (1.0, 2.0 ** -1.5, 0.25)
    for s_idx in range(3):
        for kk in range(2):
            o = 20 * 128 + (s_idx * 2 + kk) * 3
            c16[:, o + s_idx] = W4[0, kk * 128 : (kk + 1) * 128] * sscale[s_idx]

    shared = {"c32": c32, "c16": c16}
    in_maps = []
    for c in range(NCORES):
        m = dict(shared)
        xwc = np.zeros((2, NLOC + H), f)
        xwc[:, :NLOC] = x[c * NLOC : (c + 1) * NLOC].T
        xwc[:, NLOC:] = w0ts
        m["xw"] = xwc
        in_maps.append(m)
    return in_maps


def _decode_out(o):
    """[128, 12*PPH] device layout -> (U, F, Ft) flat [NLOC] arrays."""
    a = o.reshape(128, 4, 3, PPH)
    res = []
    for s_idx in range(3):
        res.append(a[:, :, s_idx, :].transpose(1, 0, 2).reshape(NLOC))
    return res


def run(inputs, trace=False):
    nc = _get_nc()
    in_maps = _prep_in_maps(inputs)
    kw = {}
    if trace:
        kw["tmpdir"] = tempfile.mkdtemp(prefix="bassk_prof_")
    res = run_bass_kernel_spmd(
        nc, in_maps, core_ids=list(range(NCORES)), trace=trace, **kw
    )
    U = np.empty((N,), np.float32)
    F = np.empty((N,), np.float32)
    Ft = np.empty((N,), np.float32)
    for c in range(NCORES):
        u, ff, ft = _decode_out(res.results[c]["out"])
        U[c * NLOC : (c + 1) * NLOC] = u
        F[c * NLOC : (c + 1) * NLOC] = ff
        Ft[c * NLOC : (c + 1) * NLOC] = ft
    shp = (B, S, 1)
    return (U.reshape(shp), F.reshape(shp), Ft.reshape(shp)), res


def kernel(**inputs):
    outs, _ = run(inputs, trace=False)
    return outs


# ---------------------------------------------------------------------------
# Dev-loop timing: persistent jitted executable (mirrors
# bass2jax.run_bass_via_pjrt's multi-core branch) so repeated executions
# reuse one compiled NEFF and can be timed back-to-back.
# ---------------------------------------------------------------------------
def _make_runner():
    if "runner" in _STATE:
        return _STATE["runner"]
    import jax
    from jax.experimental.shard_map import shard_map
    from jax.sharding import Mesh, PartitionSpec
    from concourse import bass2jax

    bass2jax.install_neuronx_cc_hook()
    nc = _get_nc()

    in_names, out_names, out_avals, zero_outs = [], [], [], []
    for alloc in nc.m.functions[0].allocations:
        if not isinstance(alloc, mybir.MemoryLocationSet):
            continue
        name = alloc.memorylocations[0].name
        if alloc.kind == "ExternalInput":
            if nc.partition_id_tensor is None or name != nc.partition_id_tensor.name:
                in_names.append(name)
        elif alloc.kind == "ExternalOutput":
            out_names.append(name)
            shape = tuple(alloc.tensor_shape)
            dtype = mybir.dt.np(alloc.dtype)
            out_avals.append(jax.core.ShapedArray(shape, dtype))
            zero_outs.append(np.zeros(shape, dtype))
    n_params = len(in_names)
    n_outs = len(out_avals)
    all_names = in_names + out_names
    if nc.partition_id_tensor is not None:
        all_names = all_names + [nc.partition_id_tensor.name]

    def _body(*args):
        operands = list(args)
        if nc.partition_id_tensor is not None:
            operands.append(bass2jax.partition_id_tensor())
        outs = bass2jax._bass_exec_p.bind(
            *operands,
            out_avals=tuple(out_avals),
            in_names=tuple(all_names),
            out_names=tuple(out_names),
            lowering_input_output_aliases=(),
            sim_require_finite=True,
            sim_require_nnan=True,
            nc=nc,
        )
        return tuple(outs)

    devices = jax.devices()[:NCORES]
    mesh = Mesh(np.asarray(devices), ("core",))
    donate = tuple(range(n_params, n_params + n_outs))
    sharded = jax.jit(
        shard_map(
            _body,
            mesh=mesh,
            in_specs=(PartitionSpec("core"),) * (n_params + n_outs),
            out_specs=(PartitionSpec("core"),) * n_outs,
            check_rep=False,
        ),
        donate_argnums=donate,
        keep_unused=True,
    )
    _STATE["runner"] = (sharded, in_names, out_names, out_avals, zero_outs)
    return _STATE["runner"]


def run_timed(inputs, iters=20):
    """Run via a persistent executable; return (outputs, per_iter_ns)."""
    import time as _time

    import jax

    sharded, in_names, out_names, out_avals, zero_outs = _make_runner()
    in_maps = _prep_in_maps(inputs)
    concat_in = [
        np.concatenate([np.asarray(in_maps[c][n]) for c in range(NCORES)], axis=0)
        for n in in_names
    ]
    dev_in = [jax.device_put(a) for a in concat_in]

    def zeros():
        return [
            np.zeros((NCORES * z.shape[0], *z.shape[1:]), z.dtype) for z in zero_outs
        ]

    # warmup (compiles on first call)
    outs = sharded(*dev_in, *zeros())
    jax.block_until_ready(outs)
    out_np = [np.asarray(o) for o in outs]

    zbufs = [zeros() for _ in range(iters)]
    t0 = _time.perf_counter()
    last = None
    for i in range(iters):
        last = sharded(*dev_in, *zbufs[i])
    jax.block_until_ready(last)
    t1 = _time.perf_counter()
    per_iter_ns = (t1 - t0) / iters * 1e9

    per_core = [
        {
            name: out_np[i].reshape(NCORES, *out_avals[i].shape)[c]
            for i, name in enumerate(out_names)
        }
        for c in range(NCORES)
    ]
    U = np.empty((N,), np.float32)
    F = np.empty((N,), np.float32)
    Ft = np.empty((N,), np.float32)
    for c in range(NCORES):
        u, ff, ft = _decode_out(per_core[c]["out"])
        U[c * NLOC : (c + 1) * NLOC] = u
        F[c * NLOC : (c + 1) * NLOC] = ff
        Ft[c * NLOC : (c + 1) * NLOC] = ft
    shp = (B, S, 1)
    return (U.reshape(shp), F.reshape(shp), Ft.reshape(shp)), per_iter_ns
